# revision 31
# baseline (speedup 1.0000x reference)
"""Trainium2 Bass kernel for CausalSelfAttention with LoRA (B=4, S=2048,
D=1024, H=16, Dh=64, rank=16), sharded over 8 NeuronCores.

Sharding: batch (4-way) x head-group (2-way). Core c handles batch c//2 and
heads (c%2)*8 .. (c%2)*8+7 (512 of the 1024 channels). Each core computes its
partial output projection; the host sums the two partials per batch element.

Host-side prep (free w.r.t. device time):
  - LoRA folded into the weights: W_eff = W + (1/rank) * b @ a  (fp64).
  - Weights/activations pre-transposed + cast to bf16 in the exact SBUF
    layouts the kernel wants.
  - 1/sqrt(Dh) folded into the Q projection weights.

Device algorithm (per core), all matmuls bf16 with fp32 PSUM accumulate:
  QT = WqT.T @ xT   [512ch, 2048tok] (transposed layout, ch on partitions)
  KT likewise; V = xT.T @ WvT [2048tok, 8 heads, 64V+64ones] (token-major;
  the 64 ones columns make the ctx matmul M=128, replicating the softmax
  denominator across psum rows 64..127 at zero extra PE cycles).
  Per head-pair, per 512-wide q block, loop over 128-wide k tiles (causal
  lower-triangle only), software-pipelined two tiles deep:
    scoresT[k, q] = KT_h.T @ QT_h     (two heads row-packed in the PE array)
    attnT = exp(scoresT)  on ScalarE (scores bounded ~|4|, no max needed)
    diagonal tiles: multiply by triangular 0/1 mask on VectorE
    [ctx ; den] += [V_h | 1].T @ attnT  (M=128: rows 0-63 ctx, 64-127 den)
  normalize: 1/den = exp(-ln(den)) on ScalarE directly on the replicated
  psum rows (64 partitions); ctxT = ctx_psum * rec on DVE. No DRAM bounce.
  out_partial = ctxT.T @ WoT          (q-major, bf16, DMA'd to HBM)

Schedule: just-in-time DMA priority order; projection work (v_proj, later
qk chunks, output projections) woven between attention kt-tiles as PE
filler so the PE never idles while ScalarE catches up on exp, keeping the
HAM clock gate at 8/8.
"""

import os
import sys

sys.path.insert(0, "/opt/trn_rl_repo")

import numpy as np
import ml_dtypes

bf16np = ml_dtypes.bfloat16

D, H, Dh, R = 1024, 16, 64, 16
S, B = 2048, 4
SCALING = 1.0 / R
N_CORES = 8

_compiled = {}


def _build_nc(fix_waits=True):
    import concourse.bass as bass
    import concourse.tile as tile
    from concourse import mybir

    fp32 = mybir.dt.float32
    bf16 = mybir.dt.bfloat16

    nc = bass.Bass()

    # xt: [128, tb, k, 512] token-block major so qk/v consumers gate on the
    # token blocks they actually touch.
    xt_d = nc.dram_tensor("xt", [128, 4, 8, 512], bf16, kind="ExternalInput")
    # wqt/wkt: [128, p, k, 128] p-chunk major (qk_proj(p) gates on chunk p).
    wqt_d = nc.dram_tensor("wqt", [128, 4, 8, 128], bf16, kind="ExternalInput")
    wkt_d = nc.dram_tensor("wkt", [128, 4, 8, 128], bf16, kind="ExternalInput")
    wvt_d = nc.dram_tensor("wvt", [128, 8, 512], bf16, kind="ExternalInput")
    wot_d = nc.dram_tensor("wot", [128, 4, D], bf16, kind="ExternalInput")
    tri_d = nc.dram_tensor("tri", [128, 2, 128], bf16, kind="ExternalInput")
    out_d = nc.dram_tensor("out", [16, 128, D], bf16, kind="ExternalOutput")

    with tile.TileContext(nc) as tc:
        with (
            tc.tile_pool(name="consts", bufs=1) as consts,
            tc.tile_pool(name="acts", bufs=1) as acts,
            tc.tile_pool(name="attn", bufs=4) as attn_pool,
            tc.tile_pool(name="small", bufs=2) as small,
            tc.tile_pool(name="ostage", bufs=3) as ostage,
            tc.tile_pool(name="ps_sc", bufs=2, space="PSUM") as ps_sc,
            tc.tile_pool(name="ps_ctx", bufs=2, space="PSUM") as ps_ctx,
        ):
            # ---- DMAs in consumption-priority order ----
            wqt = consts.tile([128, 4, 8, 128], bf16, tag="wqt")
            wkt = consts.tile([128, 4, 8, 128], bf16, tag="wkt")
            xt = consts.tile([128, 4, 8, 512], bf16, tag="xt")
            wvt = consts.tile([128, 8, 512], bf16, tag="wvt")

            tri2 = consts.tile([128, 2, 128], bf16, tag="tri")
            nc.sync.dma_start(out=tri2, in_=tri_d[:])
            nc.sync.dma_start(out=wqt[:, 0], in_=wqt_d[:, 0])
            nc.sync.dma_start(out=xt[:, 0, 0:4], in_=xt_d[:, 0, 0:4])
            nc.sync.dma_start(out=xt[:, 0, 4:8], in_=xt_d[:, 0, 4:8])
            nc.sync.dma_start(out=wkt[:, 0], in_=wkt_d[:, 0])
            nc.sync.dma_start(out=wvt, in_=wvt_d[:])
            for tb in range(1, 4):
                nc.sync.dma_start(out=xt[:, tb], in_=xt_d[:, tb])
                nc.sync.dma_start(out=wqt[:, tb], in_=wqt_d[:, tb])
                nc.sync.dma_start(out=wkt[:, tb], in_=wkt_d[:, tb])
            wot = consts.tile([128, 4, D], bf16, tag="wot")
            nc.sync.dma_start(out=wot, in_=wot_d[:])

            warm = consts.tile([128, 512], bf16, tag="warm")
            nc.vector.memset(warm, 0.5)

            qt = acts.tile([128, 4, S], bf16, tag="qt")
            ktt = acts.tile([128, 4, S], bf16, tag="ktt")
            # V with 64 ones columns per head: [tok, tile, head, 64V + 64ones]
            # so the ctx matmul (M=128) replicates the softmax denominator
            # across psum rows 64..127 for free.
            v = acts.tile([128, 16, 8, 128], bf16, tag="v")
            nc.gpsimd.memset(v[:, :, :, 64:128], 1.0)
            ctxt = acts.tile([128, 4, S], bf16, tag="ctxt")

            # ---- PE warm-up: junk matmuls while the first DMAs land, so the
            # HAM clock gate ramps before real work ----
            warm_t = ps_ctx.tile([128, 2, 512], fp32, tag="ctx", name="warm_ps")
            warm_ps = warm_t[:, 0, :]

            def warm_mm(n):
                for _ in range(n):
                    nc.tensor.matmul(
                        warm_ps[0:64, :],
                        warm[:, 0:64],
                        warm,
                        start=True,
                        stop=True,
                        skip_group_check=True,
                    )

            warm_mm(16)

            def v_proj(tt):
                # V projection for one token tile (all channel groups at once)
                vps_t = ps_sc.tile([128, 2, 512], fp32, tag="sc", name="vps")
                ps = vps_t[:, 0, :]
                tb, sub = tt // 4, tt % 4
                for k in range(8):
                    nc.tensor.matmul(
                        ps,
                        xt[:, tb, k, sub * 128:(sub + 1) * 128],
                        wvt[:, k, :],
                        start=(k == 0),
                        stop=(k == 7),
                    )
                nc.vector.tensor_copy(
                    v[:, tt, :, 0:64], ps.rearrange("p (h d) -> p h d", h=8)
                )

            def qk_tb(p, tb):
                # one token-block worth of Q+K projection for head-pair group p
                ps_t = ps_sc.tile([128, 2, 512], fp32, tag="sc", name="qk_ps")
                for k in range(8):
                    nc.tensor.matmul(
                        ps_t[:, 0, :],
                        wqt[:, p, k, :],
                        xt[:, tb, k, :],
                        start=(k == 0),
                        stop=(k == 7),
                    )
                for k in range(8):
                    nc.tensor.matmul(
                        ps_t[:, 1, :],
                        wkt[:, p, k, :],
                        xt[:, tb, k, :],
                        start=(k == 0),
                        stop=(k == 7),
                    )
                nc.vector.tensor_copy(qt[:, p, tb * 512:(tb + 1) * 512], ps_t[:, 0, :])
                nc.vector.tensor_copy(ktt[:, p, tb * 512:(tb + 1) * 512], ps_t[:, 1, :])

            def qk_half(p, tb, dst, w):
                # one self-contained half (q or k) of a qk projection token
                # block: alloc -> 8 matmuls -> copy out, psum freed at end
                def go():
                    ps_t = ps_sc.tile([128, 512], fp32, tag="sc", name="qkh_ps")
                    for k in range(8):
                        nc.tensor.matmul(
                            ps_t,
                            w[:, p, k, :],
                            xt[:, tb, k, :],
                            start=(k == 0),
                            stop=(k == 7),
                        )
                    nc.vector.tensor_copy(dst[:, p, tb * 512:(tb + 1) * 512], ps_t)

                return go

            def qk_tb_halves(p, tb):
                return [qk_half(p, tb, qt, wqt), qk_half(p, tb, ktt, wkt)]

            def oproj_half(qt_i, db):
                # one self-contained output-projection half-tile:
                # alloc -> 4 matmuls -> copy -> DMA, psum freed at end
                def go():
                    ops_t = ps_sc.tile([128, 512], fp32, tag="sc", name="op_ps")
                    for gg in range(4):
                        nc.tensor.matmul(
                            ops_t,
                            ctxt[:, gg, qt_i * 128:(qt_i + 1) * 128],
                            wot[:, gg, db * 512:(db + 1) * 512],
                            start=(gg == 0),
                            stop=(gg == 3),
                        )
                    st = ostage.tile([128, 512], bf16, tag="ostage")
                    nc.vector.tensor_copy(st, ops_t)
                    nc.sync.dma_start(
                        out=out_d[qt_i, :, db * 512:(db + 1) * 512], in_=st
                    )

                return go

            def oproj_halves(qt_i):
                return [oproj_half(qt_i, 0), oproj_half(qt_i, 1)]

            # ---- merged attention stream ----
            # All 16 (pair, q-block) attention blocks run as ONE continuous
            # depth-2 software pipeline over kt tiles: scores run two tiles
            # ahead of the exp -> (mask) -> ctx chain ACROSS block boundaries,
            # so the pipeline never cold-starts. Per-block normalization is
            # emitted as four half-size ScalarE chunks + one DVE chunk,
            # staggered one per pipeline step right after an exp, letting the
            # per-tile slack absorb each small bubble. Projection work
            # (v_proj, later qk chunks, output projections) is woven in as PE
            # filler at explicit per-block positions that respect both
            # data readiness and just-in-time need.

            def finish_parts(p, qb, ctx2):
                # normalization closures: denominator sits replicated on psum
                # rows 64..127 (one copy per head-slot in the free dim);
                # 1/den = exp(-ln(den)) on ScalarE in half-size chunks, then
                # scale ctx into bf16 ctxt on DVE (head-slot 1 written with a
                # +64 partition shift). No DRAM bounce, no broadcast.
                ld = small.tile([64, 2, 512], fp32, tag="ld")
                rec = small.tile([64, 2, 512], fp32, tag="rec")

                def ln_part(s):
                    def go():
                        nc.scalar.activation(
                            out=ld[:, s, :],
                            in_=ctx2[64:128, s, :],
                            func=mybir.ActivationFunctionType.Ln,
                        )

                    return go

                def exp_full():
                    nc.scalar.activation(
                        out=rec, in_=ld,
                        func=mybir.ActivationFunctionType.Exp, scale=-1.0,
                    )

                def muls():
                    qs = slice(qb * 512, (qb + 1) * 512)
                    nc.vector.tensor_mul(
                        ctxt[0:64, p, qs], ctx2[0:64, 0, :], rec[:, 0, :]
                    )
                    nc.vector.tensor_mul(
                        ctxt[64:128, p, qs], ctx2[0:64, 1, :], rec[:, 1, :]
                    )

                return [ln_part(0), ln_part(1), exp_full, muls]

            def run_stream(blocks):
                # blocks: list of (p, qb, fillers) with fillers a list of
                # (kt_pos, closure); pops run before the ctx matmul of that kt.
                seq = []
                fill_q = []
                for bi, (p, qb, fillers) in enumerate(blocks):
                    for kt in range(4 * (qb + 1)):
                        seq.append((bi, kt))
                    fill_q.append(sorted(fillers, key=lambda x: x[0]))
                sc_tiles = {}
                at_tiles = {}
                ctx2s = {}
                finish_q = []

                def scores(bi, kt):
                    p, qb, _ = blocks[bi]
                    j = kt - 4 * qb
                    c0 = 128 * j if j >= 0 else 0
                    sc = ps_sc.tile([128, 2, 512], fp32, tag="sc")
                    sc_tiles[(bi, kt)] = (sc, c0)
                    for s in range(2):
                        hp = slice(s * 64, (s + 1) * 64)
                        nc.tensor.matmul(
                            sc[:, s, c0:],
                            ktt[hp, p, kt * 128:(kt + 1) * 128],
                            qt[hp, p, qb * 512 + c0:(qb + 1) * 512],
                            start=True,
                            stop=True,
                            tile_position=(s * 64, 0),
                        )

                def exp_mask(bi, kt):
                    p, qb, _ = blocks[bi]
                    sc, c0 = sc_tiles.pop((bi, kt))
                    at = attn_pool.tile([128, 2, 512], bf16, tag="at")
                    at_tiles[(bi, kt)] = (at, c0)
                    nc.scalar.activation(
                        out=at[:, :, c0:],
                        in_=sc[:, :, c0:],
                        func=mybir.ActivationFunctionType.Exp,
                    )
                    if kt - 4 * qb >= 0:
                        nc.gpsimd.tensor_mul(
                            at[:, :, c0:c0 + 128], at[:, :, c0:c0 + 128], tri2
                        )

                def ctx_den(bi, kt):
                    p, qb, _ = blocks[bi]
                    if kt == 0:
                        ctx2s[bi] = ps_ctx.tile(
                            [128, 2, 512], fp32, tag="ctx", name="ctx2"
                        )
                    ctx2 = ctx2s[bi]
                    at, c0 = at_tiles.pop((bi, kt))
                    last = kt == 4 * (qb + 1) - 1
                    for s in range(2):
                        nc.tensor.matmul(
                            ctx2[:, s, c0:],
                            v[:, kt, p * 2 + s, :],
                            at[:, s, c0:],
                            start=(kt == 0),
                            stop=last,
                            skip_group_check=True,
                            tile_position=(0, 0),
                        )
                    if last:
                        finish_q.extend(finish_parts(p, qb, ctx2))

                for j in range(min(2, len(seq))):
                    scores(*seq[j])
                    exp_mask(*seq[j])
                for i, (bi, kt) in enumerate(seq):
                    if i + 2 < len(seq):
                        scores(*seq[i + 2])
                        exp_mask(*seq[i + 2])
                    if finish_q:
                        finish_q.pop(0)()
                    fq = fill_q[bi]
                    while fq and fq[0][0] <= kt:
                        fq.pop(0)[1]()
                    ctx_den(bi, kt)
                for fq in fill_q:
                    while fq:
                        fq.pop(0)[1]()
                while finish_q:
                    finish_q.pop(0)()

            # ---- schedule ----
            def vp(t):
                return lambda tt=t: v_proj(tt)

            qk_tb(0, 0)
            warm_mm(3)
            blocks = []
            # Pair 0: jit v_proj tiles + next qk token-blocks as filler.
            for qb in range(4):
                nxt = qk_tb_halves(0, qb + 1) if qb < 3 else qk_tb_halves(1, 0)
                fills = [(1, nxt[0]), (3, nxt[1])]
                fills += [(4 * qb + j, vp(4 * qb + j)) for j in range(4)]
                blocks.append((0, qb, fills))
            # Pair 1: jit next pair-1 qk + pair-2 qk chunks.
            qk12 = {qb: qk_tb_halves(1, qb + 1) for qb in range(3)}
            qk2 = {qb: qk_tb_halves(2, qb) for qb in range(4)}
            qk30 = qk_tb_halves(3, 0)
            blocks.append((1, 0, [(0, qk12[0][0]), (1, qk12[0][1]),
                                  (2, qk2[0][0]), (3, qk2[0][1])]))
            blocks.append((1, 1, [(1, qk12[1][0]), (3, qk12[1][1]),
                                  (5, qk2[1][0]), (6, qk2[1][1])]))
            blocks.append((1, 2, [(1, qk12[2][0]), (3, qk12[2][1]),
                                  (6, qk2[2][0]), (8, qk2[2][1])]))
            blocks.append((1, 3, [(1, qk2[3][0]), (4, qk2[3][1]),
                                  (8, qk30[0]), (11, qk30[1])]))
            # Pairs 2+3 descending; pair-3 qk chunks then finished blocks'
            # output projections as filler, placed so the target block's
            # staggered normalization has completed.
            qk31 = qk_tb_halves(3, 1)
            qk32 = qk_tb_halves(3, 2)
            qk33 = qk_tb_halves(3, 3)
            blocks.append((2, 3, [(1, qk31[0]), (3, qk31[1]),
                                  (6, qk33[0]), (9, qk33[1])]))
            blocks.append((3, 3, [(2, qk32[0]), (4, qk32[1])]))
            op = {t: oproj_halves(t) for t in range(16)}
            blocks.append((2, 2, [(5, op[12][0]), (7, op[12][1]),
                                  (9, op[13][0]), (11, op[13][1])]))
            blocks.append((3, 2, [(1, op[14][0]), (3, op[14][1]),
                                  (5, op[15][0]), (7, op[15][1])]))
            blocks.append((2, 1, [(4, op[8][0]), (5, op[8][1]),
                                  (6, op[9][0]), (7, op[9][1])]))
            blocks.append((3, 1, [(0, op[10][0]), (1, op[10][1]),
                                  (2, op[11][0]), (3, op[11][1])]))
            blocks.append((2, 0, []))
            blocks.append((3, 0, [(1, op[4][0]), (1, op[4][1]),
                                  (2, op[5][0]), (2, op[5][1]),
                                  (3, op[6][0]), (3, op[6][1]),
                                  (9, op[7][0]), (9, op[7][1])]))
            run_stream(blocks)
            for qt_i in range(0, 4):
                for f in op[qt_i]:
                    f()

    if fix_waits:
        _fix_matmul_waits(nc, mybir)
    return nc


_WAIT_LIMITS = {"InstISA": 0}


def _fix_matmul_waits(nc, mybir):
    """Walrus encodes at most one sync-wait command on compute-engine datapath
    instructions (MM/TT/ACT/...), and none at all on InstISA (incl. custom DVE
    ops, which also can't carry sem updates). Split excess waits into
    standalone InstEventSemaphore waits immediately before, and ISA updates
    into a standalone update immediately after — semantically identical
    (same engine stream, same point)."""
    import bass_rust

    counter = [0]

    def make_ev(engine, waits, updates):
        counter[0] += 1
        ev = mybir.InstEventSemaphore(name=f"W-split-{counter[0]}", ins=[], outs=[])
        ev.engine = engine
        ev.sync_info = bass_rust.SyncInfo(on_wait=waits, on_update=updates)
        return ev

    for blk in nc.m.functions[0].blocks:
        insts = list(blk.instructions)
        out = []
        changed = False
        for ins in insts:
            si = ins.sync_info
            is_isa = isinstance(ins, mybir.InstISA)
            limit = 0 if is_isa else _WAIT_LIMITS.get(type(ins).__name__, 1)
            post = None
            if si is not None and (
                len(si.on_wait) > limit or (is_isa and si.on_update)
            ):
                waits = list(si.on_wait)
                if limit:
                    extra, keep = waits[:-limit], waits[-limit:]
                else:
                    extra, keep = waits, []
                for w in extra:
                    out.append(make_ev(ins.engine, [w], []))
                si.on_wait = keep
                if is_isa and si.on_update:
                    post = make_ev(ins.engine, [], list(si.on_update))
                    si.on_update = []
                ins.sync_info = si
                changed = True
            out.append(ins)
            if post is not None:
                out.append(post)
        if changed:
            blk.instructions = out


def _get_nc():
    if "nc" not in _compiled:
        _compiled["nc"] = _build_nc()
    return _compiled["nc"]


def _fold(w, a, b):
    return w.astype(np.float64) + SCALING * (
        b.astype(np.float64) @ a.astype(np.float64)
    )


def _prep_in_maps(inputs):
    x = np.asarray(inputs["x"], np.float32)
    wq_e = _fold(inputs["wq"], inputs["aq"], inputs["bq"])
    wk_e = _fold(inputs["wk"], inputs["ak"], inputs["bk"])
    wv_e = _fold(inputs["wv"], inputs["av"], inputs["bv"])
    wo_e = _fold(inputs["wo"], inputs["ao"], inputs["bo"])

    tri = np.triu(np.ones((128, 128), np.float32)).astype(bf16np)
    tri2 = np.ascontiguousarray(np.broadcast_to(tri[:, None, :], (128, 2, 128)))

    in_maps = []
    for c in range(N_CORES):
        b, g = c // 2, c % 2
        gs = slice(g * 512, (g + 1) * 512)
        # xt: [128, tb, k, 512]
        xt = (
            x[b].T.reshape(8, 128, 4, 512).transpose(1, 2, 0, 3).astype(bf16np)
        )
        # wqt/wkt: [128, p, k, 128]
        wqt = (
            (wq_e[gs].T * 0.125)
            .reshape(8, 128, 4, 128)
            .transpose(1, 2, 0, 3)
            .astype(bf16np)
        )
        wkt = wk_e[gs].T.reshape(8, 128, 4, 128).transpose(1, 2, 0, 3).astype(bf16np)
        wvt = wv_e[gs].T.reshape(8, 128, 512).transpose(1, 0, 2).astype(bf16np)
        wot = wo_e[:, gs].T.reshape(4, 128, D).transpose(1, 0, 2).astype(bf16np)
        in_maps.append(
            dict(
                xt=np.ascontiguousarray(xt),
                wqt=np.ascontiguousarray(wqt),
                wkt=np.ascontiguousarray(wkt),
                wvt=np.ascontiguousarray(wvt),
                wot=np.ascontiguousarray(wot),
                tri=tri2,
            )
        )
    return in_maps


def run(inputs, trace=False, **kw):
    """Run on 8 cores; returns (full_output, BassKernelResults)."""
    from concourse.bass_utils import run_bass_kernel_spmd

    nc = _get_nc()
    in_maps = _prep_in_maps(inputs)
    res = run_bass_kernel_spmd(
        nc, in_maps, core_ids=list(range(N_CORES)), trace=trace, **kw
    )
    full = np.zeros((B, S, D), np.float32)
    for b in range(B):
        o0 = np.asarray(res.results[2 * b]["out"]).astype(np.float32).reshape(S, D)
        o1 = np.asarray(res.results[2 * b + 1]["out"]).astype(np.float32).reshape(S, D)
        full[b] = o0 + o1
    return full, res


def kernel(**inputs):
    full, _ = run(inputs, trace=False)
    return full


# revision 33
# speedup vs baseline: 1.0186x; 1.0186x over previous
"""Trainium2 Bass kernel for CausalSelfAttention with LoRA (B=4, S=2048,
D=1024, H=16, Dh=64, rank=16), sharded over 8 NeuronCores.

Sharding: batch (4-way) x head-group (2-way). Core c handles batch c//2 and
heads (c%2)*8 .. (c%2)*8+7 (512 of the 1024 channels). Each core computes its
partial output projection; the host sums the two partials per batch element.

Host-side prep (free w.r.t. device time):
  - LoRA folded into the weights: W_eff = W + (1/rank) * b @ a  (fp64).
  - Weights/activations pre-transposed + cast to bf16 in the exact SBUF
    layouts the kernel wants.
  - 1/sqrt(Dh) folded into the Q projection weights.

Device algorithm (per core), all matmuls bf16 with fp32 PSUM accumulate:
  QT = WqT.T @ xT   [512ch, 2048tok] (transposed layout, ch on partitions)
  KT likewise; V = xT.T @ WvT [2048tok, 8 heads, 64V+64ones] (token-major;
  the 64 ones columns make the ctx matmul M=128, replicating the softmax
  denominator across psum rows 64..127 at zero extra PE cycles).
  Per head-pair, per 512-wide q block, loop over 128-wide k tiles (causal
  lower-triangle only), software-pipelined two tiles deep:
    scoresT[k, q] = KT_h.T @ QT_h     (two heads row-packed in the PE array)
    attnT = exp(scoresT)  on ScalarE (scores bounded ~|4|, no max needed)
    diagonal tiles: multiply by triangular 0/1 mask on VectorE
    [ctx ; den] += [V_h | 1].T @ attnT  (M=128: rows 0-63 ctx, 64-127 den)
  normalize: 1/den = exp(-ln(den)) on ScalarE directly on the replicated
  psum rows (64 partitions); ctxT = ctx_psum * rec on DVE. No DRAM bounce.
  out_partial = ctxT.T @ WoT          (q-major, bf16, DMA'd to HBM)

Schedule: just-in-time DMA priority order; projection work (v_proj, later
qk chunks, output projections) woven between attention kt-tiles as PE
filler so the PE never idles while ScalarE catches up on exp, keeping the
HAM clock gate at 8/8.
"""

import os
import sys

sys.path.insert(0, "/opt/trn_rl_repo")

import numpy as np
import ml_dtypes

bf16np = ml_dtypes.bfloat16

D, H, Dh, R = 1024, 16, 64, 16
S, B = 2048, 4
SCALING = 1.0 / R
N_CORES = 8

_compiled = {}


def _build_nc(fix_waits=True):
    import concourse.bass as bass
    import concourse.tile as tile
    from concourse import mybir

    fp32 = mybir.dt.float32
    bf16 = mybir.dt.bfloat16

    nc = bass.Bass()

    # xt: [128, tb, k, 512] token-block major so qk/v consumers gate on the
    # token blocks they actually touch.
    xt_d = nc.dram_tensor("xt", [128, 4, 8, 512], bf16, kind="ExternalInput")
    # wqt/wkt: [128, p, k, 128] p-chunk major (qk_proj(p) gates on chunk p).
    wqt_d = nc.dram_tensor("wqt", [128, 4, 8, 128], bf16, kind="ExternalInput")
    wkt_d = nc.dram_tensor("wkt", [128, 4, 8, 128], bf16, kind="ExternalInput")
    wvt_d = nc.dram_tensor("wvt", [128, 8, 512], bf16, kind="ExternalInput")
    wot_d = nc.dram_tensor("wot", [128, 4, D], bf16, kind="ExternalInput")
    tri_d = nc.dram_tensor("tri", [128, 2, 128], bf16, kind="ExternalInput")
    out_d = nc.dram_tensor("out", [16, 128, D], bf16, kind="ExternalOutput")

    with tile.TileContext(nc) as tc:
        with (
            tc.tile_pool(name="consts", bufs=1) as consts,
            tc.tile_pool(name="acts", bufs=1) as acts,
            tc.tile_pool(name="attn", bufs=4) as attn_pool,
            tc.tile_pool(name="small", bufs=2) as small,
            tc.tile_pool(name="ostage", bufs=3) as ostage,
            tc.tile_pool(name="ps_sc", bufs=2, space="PSUM") as ps_sc,
            tc.tile_pool(name="ps_ctx", bufs=2, space="PSUM") as ps_ctx,
        ):
            # ---- DMAs in consumption-priority order ----
            wqt = consts.tile([128, 4, 8, 128], bf16, tag="wqt")
            wkt = consts.tile([128, 4, 8, 128], bf16, tag="wkt")
            xt = consts.tile([128, 4, 8, 512], bf16, tag="xt")
            wvt = consts.tile([128, 8, 512], bf16, tag="wvt")

            tri2 = consts.tile([128, 2, 128], bf16, tag="tri")
            nc.sync.dma_start(out=tri2, in_=tri_d[:])
            nc.sync.dma_start(out=wqt[:, 0], in_=wqt_d[:, 0])
            nc.sync.dma_start(out=xt[:, 0, 0:4], in_=xt_d[:, 0, 0:4])
            nc.sync.dma_start(out=xt[:, 0, 4:8], in_=xt_d[:, 0, 4:8])
            nc.sync.dma_start(out=wkt[:, 0], in_=wkt_d[:, 0])
            nc.sync.dma_start(out=wvt, in_=wvt_d[:])
            for tb in range(1, 4):
                nc.sync.dma_start(out=xt[:, tb], in_=xt_d[:, tb])
                nc.sync.dma_start(out=wqt[:, tb], in_=wqt_d[:, tb])
                nc.sync.dma_start(out=wkt[:, tb], in_=wkt_d[:, tb])
            wot = consts.tile([128, 4, D], bf16, tag="wot")
            nc.sync.dma_start(out=wot, in_=wot_d[:])

            warm = consts.tile([128, 512], bf16, tag="warm")
            nc.vector.memset(warm, 0.5)

            qt = acts.tile([128, 4, S], bf16, tag="qt")
            ktt = acts.tile([128, 4, S], bf16, tag="ktt")
            # V with 64 ones columns per head: [tok, tile, head, 64V + 64ones]
            # so the ctx matmul (M=128) replicates the softmax denominator
            # across psum rows 64..127 for free.
            v = acts.tile([128, 16, 8, 128], bf16, tag="v")
            nc.gpsimd.memset(v[:, :, :, 64:128], 1.0)
            ctxt = acts.tile([128, 4, S], bf16, tag="ctxt")

            # ---- PE warm-up: junk matmuls while the first DMAs land, so the
            # HAM clock gate ramps before real work ----
            warm_t = ps_ctx.tile([128, 2, 512], fp32, tag="ctx", name="warm_ps")
            warm_ps = warm_t[:, 0, :]

            def warm_mm(n):
                for _ in range(n):
                    nc.tensor.matmul(
                        warm_ps[0:64, :],
                        warm[:, 0:64],
                        warm,
                        start=True,
                        stop=True,
                        skip_group_check=True,
                    )

            warm_mm(16)

            def v_proj(tt):
                # V projection for one token tile (all channel groups at once)
                vps_t = ps_sc.tile([128, 2, 512], fp32, tag="sc", name="vps")
                ps = vps_t[:, 0, :]
                tb, sub = tt // 4, tt % 4
                for k in range(8):
                    nc.tensor.matmul(
                        ps,
                        xt[:, tb, k, sub * 128:(sub + 1) * 128],
                        wvt[:, k, :],
                        start=(k == 0),
                        stop=(k == 7),
                    )
                nc.vector.tensor_copy(
                    v[:, tt, :, 0:64], ps.rearrange("p (h d) -> p h d", h=8)
                )

            def qk_tb(p, tb):
                # one token-block worth of Q+K projection for head-pair group p
                ps_t = ps_sc.tile([128, 2, 512], fp32, tag="sc", name="qk_ps")
                for k in range(8):
                    nc.tensor.matmul(
                        ps_t[:, 0, :],
                        wqt[:, p, k, :],
                        xt[:, tb, k, :],
                        start=(k == 0),
                        stop=(k == 7),
                    )
                for k in range(8):
                    nc.tensor.matmul(
                        ps_t[:, 1, :],
                        wkt[:, p, k, :],
                        xt[:, tb, k, :],
                        start=(k == 0),
                        stop=(k == 7),
                    )
                nc.vector.tensor_copy(qt[:, p, tb * 512:(tb + 1) * 512], ps_t[:, 0, :])
                nc.vector.tensor_copy(ktt[:, p, tb * 512:(tb + 1) * 512], ps_t[:, 1, :])

            def qk_half(p, tb, dst, w):
                # one self-contained half (q or k) of a qk projection token
                # block: alloc -> 8 matmuls -> copy out, psum freed at end
                def go():
                    ps_t = ps_sc.tile([128, 512], fp32, tag="sc", name="qkh_ps")
                    for k in range(8):
                        nc.tensor.matmul(
                            ps_t,
                            w[:, p, k, :],
                            xt[:, tb, k, :],
                            start=(k == 0),
                            stop=(k == 7),
                        )
                    nc.vector.tensor_copy(dst[:, p, tb * 512:(tb + 1) * 512], ps_t)

                return go

            def qk_tb_halves(p, tb):
                return [qk_half(p, tb, qt, wqt), qk_half(p, tb, ktt, wkt)]

            def oproj_half(qt_i, db):
                # one self-contained output-projection half-tile:
                # alloc -> 4 matmuls -> copy -> DMA, psum freed at end
                def go():
                    ops_t = ps_sc.tile([128, 512], fp32, tag="sc", name="op_ps")
                    for gg in range(4):
                        nc.tensor.matmul(
                            ops_t,
                            ctxt[:, gg, qt_i * 128:(qt_i + 1) * 128],
                            wot[:, gg, db * 512:(db + 1) * 512],
                            start=(gg == 0),
                            stop=(gg == 3),
                        )
                    st = ostage.tile([128, 512], bf16, tag="ostage")
                    nc.vector.tensor_copy(st, ops_t)
                    nc.sync.dma_start(
                        out=out_d[qt_i, :, db * 512:(db + 1) * 512], in_=st
                    )

                return go

            def oproj_halves(qt_i):
                return [oproj_half(qt_i, 0), oproj_half(qt_i, 1)]

            # ---- merged attention stream ----
            # All 16 (pair, q-block) attention blocks run as ONE continuous
            # depth-2 software pipeline over kt tiles: scores run two tiles
            # ahead of the exp -> (mask) -> ctx chain ACROSS block boundaries,
            # so the pipeline never cold-starts. Per-block normalization is
            # emitted as four half-size ScalarE chunks + one DVE chunk,
            # staggered one per pipeline step right after an exp, letting the
            # per-tile slack absorb each small bubble. Projection work
            # (v_proj, later qk chunks, output projections) is woven in as PE
            # filler at explicit per-block positions that respect both
            # data readiness and just-in-time need.

            def finish_parts(p, qb, ctx2):
                # normalization closures: denominator sits replicated on psum
                # rows 64..127 (one copy per head-slot in the free dim);
                # 1/den = exp(-ln(den)) on ScalarE in half-size chunks, then
                # scale ctx into bf16 ctxt on DVE (head-slot 1 written with a
                # +64 partition shift). No DRAM bounce, no broadcast.
                ld = small.tile([64, 2, 512], fp32, tag="ld")
                rec = small.tile([64, 2, 512], fp32, tag="rec")

                def ln_part(s):
                    def go():
                        nc.scalar.activation(
                            out=ld[:, s, :],
                            in_=ctx2[64:128, s, :],
                            func=mybir.ActivationFunctionType.Ln,
                        )

                    return go

                def exp_part(s):
                    def go():
                        nc.scalar.activation(
                            out=rec[:, s, :], in_=ld[:, s, :],
                            func=mybir.ActivationFunctionType.Exp, scale=-1.0,
                        )

                    return go

                def muls():
                    qs = slice(qb * 512, (qb + 1) * 512)
                    nc.vector.tensor_mul(
                        ctxt[0:64, p, qs], ctx2[0:64, 0, :], rec[:, 0, :]
                    )
                    nc.vector.tensor_mul(
                        ctxt[64:128, p, qs], ctx2[0:64, 1, :], rec[:, 1, :]
                    )

                return [ln_part(0), ln_part(1), exp_part(0), exp_part(1), muls]

            def run_stream(blocks):
                # blocks: list of (p, qb, fillers) with fillers a list of
                # (kt_pos, closure); pops run before the ctx matmul of that kt.
                seq = []
                fill_q = []
                for bi, (p, qb, fillers) in enumerate(blocks):
                    for kt in range(4 * (qb + 1)):
                        seq.append((bi, kt))
                    fill_q.append(sorted(fillers, key=lambda x: x[0]))
                sc_tiles = {}
                at_tiles = {}
                ctx2s = {}
                finish_q = []

                def scores(bi, kt):
                    p, qb, _ = blocks[bi]
                    j = kt - 4 * qb
                    c0 = 128 * j if j >= 0 else 0
                    sc = ps_sc.tile([128, 2, 512], fp32, tag="sc")
                    sc_tiles[(bi, kt)] = (sc, c0)
                    for s in range(2):
                        hp = slice(s * 64, (s + 1) * 64)
                        nc.tensor.matmul(
                            sc[:, s, c0:],
                            ktt[hp, p, kt * 128:(kt + 1) * 128],
                            qt[hp, p, qb * 512 + c0:(qb + 1) * 512],
                            start=True,
                            stop=True,
                            tile_position=(s * 64, 0),
                        )

                def exp_mask(bi, kt):
                    p, qb, _ = blocks[bi]
                    sc, c0 = sc_tiles.pop((bi, kt))
                    at = attn_pool.tile([128, 2, 512], bf16, tag="at")
                    at_tiles[(bi, kt)] = (at, c0)
                    nc.scalar.activation(
                        out=at[:, :, c0:],
                        in_=sc[:, :, c0:],
                        func=mybir.ActivationFunctionType.Exp,
                    )
                    if kt - 4 * qb >= 0:
                        nc.gpsimd.tensor_mul(
                            at[:, :, c0:c0 + 128], at[:, :, c0:c0 + 128], tri2
                        )

                def ctx_den(bi, kt):
                    p, qb, _ = blocks[bi]
                    if kt == 0:
                        ctx2s[bi] = ps_ctx.tile(
                            [128, 2, 512], fp32, tag="ctx", name="ctx2"
                        )
                    ctx2 = ctx2s[bi]
                    at, c0 = at_tiles.pop((bi, kt))
                    last = kt == 4 * (qb + 1) - 1
                    for s in range(2):
                        nc.tensor.matmul(
                            ctx2[:, s, c0:],
                            v[:, kt, p * 2 + s, :],
                            at[:, s, c0:],
                            start=(kt == 0),
                            stop=last,
                            skip_group_check=True,
                            tile_position=(0, 0),
                        )
                    if last:
                        finish_q.extend(finish_parts(p, qb, ctx2))

                for j in range(min(2, len(seq))):
                    scores(*seq[j])
                    exp_mask(*seq[j])
                for i, (bi, kt) in enumerate(seq):
                    if i + 2 < len(seq):
                        scores(*seq[i + 2])
                        exp_mask(*seq[i + 2])
                    if finish_q:
                        finish_q.pop(0)()
                    fq = fill_q[bi]
                    while fq and fq[0][0] <= kt:
                        fq.pop(0)[1]()
                    ctx_den(bi, kt)
                for fq in fill_q:
                    while fq:
                        fq.pop(0)[1]()
                while finish_q:
                    finish_q.pop(0)()

            # ---- schedule ----
            def vp(t):
                return lambda tt=t: v_proj(tt)

            qk_tb(0, 0)
            warm_mm(3)
            blocks = []
            # Pair 0: jit v_proj tiles + next qk token-blocks as filler.
            for qb in range(4):
                nxt = qk_tb_halves(0, qb + 1) if qb < 3 else qk_tb_halves(1, 0)
                fills = [(1, nxt[0]), (3, nxt[1])]
                fills += [(4 * qb + j, vp(4 * qb + j)) for j in range(4)]
                blocks.append((0, qb, fills))
            # Pair 1: jit next pair-1 qk + pair-2 qk chunks.
            qk12 = {qb: qk_tb_halves(1, qb + 1) for qb in range(3)}
            qk2 = {qb: qk_tb_halves(2, qb) for qb in range(4)}
            qk30 = qk_tb_halves(3, 0)
            blocks.append((1, 0, [(0, qk12[0][0]), (1, qk12[0][1]),
                                  (2, qk2[0][0]), (3, qk2[0][1])]))
            blocks.append((1, 1, [(1, qk12[1][0]), (3, qk12[1][1]),
                                  (5, qk2[1][0]), (6, qk2[1][1])]))
            blocks.append((1, 2, [(1, qk12[2][0]), (3, qk12[2][1]),
                                  (6, qk2[2][0]), (8, qk2[2][1])]))
            blocks.append((1, 3, [(1, qk2[3][0]), (4, qk2[3][1]),
                                  (8, qk30[0]), (11, qk30[1])]))
            # Pairs 2+3 descending; pair-3 qk chunks then finished blocks'
            # output projections as filler, placed so the target block's
            # staggered normalization has completed.
            qk31 = qk_tb_halves(3, 1)
            qk32 = qk_tb_halves(3, 2)
            qk33 = qk_tb_halves(3, 3)
            blocks.append((2, 3, [(1, qk31[0]), (3, qk31[1]),
                                  (6, qk33[0]), (9, qk33[1])]))
            blocks.append((3, 3, [(2, qk32[0]), (4, qk32[1])]))
            op = {t: oproj_halves(t) for t in range(16)}
            blocks.append((2, 2, [(5, op[12][0]), (7, op[12][1]),
                                  (9, op[13][0]), (11, op[13][1])]))
            blocks.append((3, 2, [(1, op[14][0]), (3, op[14][1]),
                                  (5, op[15][0]), (7, op[15][1])]))
            blocks.append((2, 1, [(4, op[8][0]), (5, op[8][1]),
                                  (6, op[9][0]), (7, op[9][1])]))
            blocks.append((3, 1, [(0, op[10][0]), (1, op[10][1]),
                                  (2, op[11][0]), (3, op[11][1])]))
            blocks.append((2, 0, []))
            blocks.append((3, 0, [(1, op[4][0]), (1, op[4][1]),
                                  (2, op[5][0]), (2, op[5][1]),
                                  (3, op[6][0]), (3, op[6][1]),
                                  (9, op[7][0]), (9, op[7][1])]))
            run_stream(blocks)
            for qt_i in range(0, 4):
                for f in op[qt_i]:
                    f()

    if fix_waits:
        _fix_matmul_waits(nc, mybir)
    return nc


_WAIT_LIMITS = {"InstISA": 0}


def _fix_matmul_waits(nc, mybir):
    """Walrus encodes at most one sync-wait command on compute-engine datapath
    instructions (MM/TT/ACT/...), and none at all on InstISA (incl. custom DVE
    ops, which also can't carry sem updates). Split excess waits into
    standalone InstEventSemaphore waits immediately before, and ISA updates
    into a standalone update immediately after — semantically identical
    (same engine stream, same point)."""
    import bass_rust

    counter = [0]

    def make_ev(engine, waits, updates):
        counter[0] += 1
        ev = mybir.InstEventSemaphore(name=f"W-split-{counter[0]}", ins=[], outs=[])
        ev.engine = engine
        ev.sync_info = bass_rust.SyncInfo(on_wait=waits, on_update=updates)
        return ev

    for blk in nc.m.functions[0].blocks:
        insts = list(blk.instructions)
        out = []
        changed = False
        for ins in insts:
            si = ins.sync_info
            is_isa = isinstance(ins, mybir.InstISA)
            limit = 0 if is_isa else _WAIT_LIMITS.get(type(ins).__name__, 1)
            post = None
            if si is not None and (
                len(si.on_wait) > limit or (is_isa and si.on_update)
            ):
                waits = list(si.on_wait)
                if limit:
                    extra, keep = waits[:-limit], waits[-limit:]
                else:
                    extra, keep = waits, []
                for w in extra:
                    out.append(make_ev(ins.engine, [w], []))
                si.on_wait = keep
                if is_isa and si.on_update:
                    post = make_ev(ins.engine, [], list(si.on_update))
                    si.on_update = []
                ins.sync_info = si
                changed = True
            out.append(ins)
            if post is not None:
                out.append(post)
        if changed:
            blk.instructions = out


def _get_nc():
    if "nc" not in _compiled:
        _compiled["nc"] = _build_nc()
    return _compiled["nc"]


def _fold(w, a, b):
    return w.astype(np.float64) + SCALING * (
        b.astype(np.float64) @ a.astype(np.float64)
    )


def _prep_in_maps(inputs):
    x = np.asarray(inputs["x"], np.float32)
    wq_e = _fold(inputs["wq"], inputs["aq"], inputs["bq"])
    wk_e = _fold(inputs["wk"], inputs["ak"], inputs["bk"])
    wv_e = _fold(inputs["wv"], inputs["av"], inputs["bv"])
    wo_e = _fold(inputs["wo"], inputs["ao"], inputs["bo"])

    tri = np.triu(np.ones((128, 128), np.float32)).astype(bf16np)
    tri2 = np.ascontiguousarray(np.broadcast_to(tri[:, None, :], (128, 2, 128)))

    in_maps = []
    for c in range(N_CORES):
        b, g = c // 2, c % 2
        gs = slice(g * 512, (g + 1) * 512)
        # xt: [128, tb, k, 512]
        xt = (
            x[b].T.reshape(8, 128, 4, 512).transpose(1, 2, 0, 3).astype(bf16np)
        )
        # wqt/wkt: [128, p, k, 128]
        wqt = (
            (wq_e[gs].T * 0.125)
            .reshape(8, 128, 4, 128)
            .transpose(1, 2, 0, 3)
            .astype(bf16np)
        )
        wkt = wk_e[gs].T.reshape(8, 128, 4, 128).transpose(1, 2, 0, 3).astype(bf16np)
        wvt = wv_e[gs].T.reshape(8, 128, 512).transpose(1, 0, 2).astype(bf16np)
        wot = wo_e[:, gs].T.reshape(4, 128, D).transpose(1, 0, 2).astype(bf16np)
        in_maps.append(
            dict(
                xt=np.ascontiguousarray(xt),
                wqt=np.ascontiguousarray(wqt),
                wkt=np.ascontiguousarray(wkt),
                wvt=np.ascontiguousarray(wvt),
                wot=np.ascontiguousarray(wot),
                tri=tri2,
            )
        )
    return in_maps


def run(inputs, trace=False, **kw):
    """Run on 8 cores; returns (full_output, BassKernelResults)."""
    from concourse.bass_utils import run_bass_kernel_spmd

    nc = _get_nc()
    in_maps = _prep_in_maps(inputs)
    res = run_bass_kernel_spmd(
        nc, in_maps, core_ids=list(range(N_CORES)), trace=trace, **kw
    )
    full = np.zeros((B, S, D), np.float32)
    for b in range(B):
        o0 = np.asarray(res.results[2 * b]["out"]).astype(np.float32).reshape(S, D)
        o1 = np.asarray(res.results[2 * b + 1]["out"]).astype(np.float32).reshape(S, D)
        full[b] = o0 + o1
    return full, res


def kernel(**inputs):
    full, _ = run(inputs, trace=False)
    return full
